# revision 1
# baseline (speedup 1.0000x reference)
"""Trainium2 Bass kernel for nn_AttnLayerV3 (differential attention layer).

Tensor-parallel over heads across 8 NeuronCores:
  - core c owns q-heads {2c, 2c+1} and kv-head c//2 (duplicated per core pair)
  - per-core: Q/K/V projections, RoPE, two-stream causal softmax attention
    (computed transposed: scores (k, q) so attention weights are directly the
    lhsT of the AV matmul), per-head GroupNorm, partial output projection
  - host: shards/permutes weights, gathers the 8 partial outputs and sums.

All matmuls run in bf16 with fp32 PSUM accumulation (verified ~7e-3 rel err
vs the fp32 reference).
"""

import numpy as np
import ml_dtypes

import concourse.bass as bass
import concourse.bacc as bacc
import concourse.tile as tile
import concourse.mybir as mybir
from concourse.bass_utils import run_bass_kernel_spmd
from concourse.masks import make_identity, make_upper_triangular

bf16 = ml_dtypes.bfloat16

B, T, D = 2, 1024, 2048
H, KV, DH = 16, 4, 128
NCORES = 8
HPC = H // NCORES          # q heads per core = 2
TOK = B * T                # 2048
LAMBDA_INIT = 0.8 - 0.6 * float(np.exp(-0.3 * 1))
GN_EPS = 1e-5
ROPE_BASE = 10000.0

KB = 16                    # contraction blocks of 128 over D
TSB = TOK // 512           # 4 token superblocks of 512
QB = T // 128              # 8 q/k blocks of 128 per batch
NTB = TOK // 128           # 16 token blocks of 128


def build_program(lam: float):
    f32 = mybir.dt.float32
    bf = mybir.dt.bfloat16
    nc = bacc.Bacc("TRN2", target_bir_lowering=False, debug=False,
                   num_devices=NCORES)

    xT_d = nc.dram_tensor("xT", (TSB, 128, KB, 512), bf, kind="ExternalInput").ap()
    wq_d = nc.dram_tensor("wqT", (4, 128, KB, 128), bf, kind="ExternalInput").ap()
    wk_d = nc.dram_tensor("wkT", (2, 128, KB, 128), bf, kind="ExternalInput").ap()
    wv_d = nc.dram_tensor("wvT", (128, KB, 256), bf, kind="ExternalInput").ap()
    wo_d = nc.dram_tensor("woT", (128, 4, D), bf, kind="ExternalInput").ap()
    tbl_d = nc.dram_tensor("tbl", (128, TOK), bf, kind="ExternalInput").ap()
    tb2_d = nc.dram_tensor("tbl2", (128, TOK), bf, kind="ExternalInput").ap()
    out_d = nc.dram_tensor("out", (TOK, D), bf, kind="ExternalOutput").ap()

    with tile.TileContext(nc) as tc:
        _body(tc, xT_d, wq_d, wk_d, wv_d, wo_d, tbl_d, tb2_d, out_d, lam)
    nc.compile()
    return nc


def _body(tc, xT_d, wq_d, wk_d, wv_d, wo_d, tbl_d, tb2_d, out_d, lam):
    nc = tc.nc
    f32 = mybir.dt.float32
    bf = mybir.dt.bfloat16
    mult = mybir.AluOpType.mult
    sub = mybir.AluOpType.subtract
    add = mybir.AluOpType.add

    import contextlib
    ctx = contextlib.ExitStack()
    with ctx:
        # ---- pools -------------------------------------------------------
        big = ctx.enter_context(tc.tile_pool(name="big", bufs=1))
        wpool = ctx.enter_context(tc.tile_pool(name="wq", bufs=6))
        tmp = ctx.enter_context(tc.tile_pool(name="tmp", bufs=2))
        rtmp = ctx.enter_context(tc.tile_pool(name="rtmp", bufs=2))
        opool = ctx.enter_context(tc.tile_pool(name="o", bufs=5))
        opool2 = ctx.enter_context(tc.tile_pool(name="o2", bufs=8))
        spool = ctx.enter_context(tc.tile_pool(name="s", bufs=5))
        epool = ctx.enter_context(tc.tile_pool(name="e", bufs=1))
        stage = ctx.enter_context(tc.tile_pool(name="stage", bufs=3))
        pmm = ctx.enter_context(tc.tile_pool(name="pmm", bufs=2, space="PSUM"))
        pav = ctx.enter_context(tc.tile_pool(name="pav", bufs=2, space="PSUM"))
        pop = ctx.enter_context(tc.tile_pool(name="pop", bufs=2, space="PSUM"))
        # (8 PSUM banks total: 2 + 4 + 2)

        # ---- constants ---------------------------------------------------
        ident = big.tile([128, 128], bf, tag="ident")
        make_identity(nc, ident[:])
        tri = big.tile([128, 128], bf, tag="tri")
        make_upper_triangular(nc, tri[:], val=1.0, diag=True)

        # weights for the two K streams first (unblock first matmuls)
        wts = {}
        wt0 = wpool.tile([128, KB, 128], bf, tag="w", name="wtk0")
        nc.sync.dma_start(wt0[:], wk_d[0])
        wts[4] = wt0

        # ---- resident tensors -------------------------------------------
        xsb = big.tile([128, KB, TOK], bf, tag="xsb")
        for kc in range(4):
            nc.sync.dma_start(xsb[:, kc * 4:(kc + 1) * 4, 0:512],
                              xT_d[0][:, kc * 4:(kc + 1) * 4])
        wt1 = wpool.tile([128, KB, 128], bf, tag="w", name="wtk1")
        nc.sync.dma_start(wt1[:], wk_d[1])
        wts[5] = wt1
        for cb in range(4):
            wt = wpool.tile([128, KB, 128], bf, tag="w", name=f"wtq{cb}")
            nc.sync.dma_start(wt[:], wq_d[cb])
            wts[cb] = wt
        tbl = big.tile([128, TOK], bf, tag="tbl")        # cos rows 0:64, sin 64:128
        nc.sync.dma_start(tbl[:], tbl_d)
        tbl2 = big.tile([128, TOK], bf, tag="tbl2")      # sin rows 0:64, cos 64:128
        nc.sync.dma_start(tbl2[:], tb2_d)
        cosv = tbl[0:64, :]      # base 0
        sinv = tbl[64:128, :]    # base 64
        sinv0 = tbl2[0:64, :]    # sin at base 0
        cosv64 = tbl2[64:128, :]  # cos at base 64
        wv = big.tile([128, KB, 256], bf, tag="wv")
        nc.sync.dma_start(wv[:], wv_d)
        for ts in range(1, TSB):
            nc.sync.dma_start(xsb[:, :, ts * 512:(ts + 1) * 512], xT_d[ts])

        # q streams (4 blocks: h0s0 h0s1 h1s0 h1s1) then k streams (2 blocks)
        qk = big.tile([128, 6, TOK], bf, tag="qk")
        vsb = big.tile([128, NTB, 257], bf, tag="vsb")
        nc.vector.memset(vsb[:, :, 256:257], 1.0)
        osbT = big.tile([128, 4, TOK], bf, tag="osbT")   # o transposed, ch-major
        wo_sb = big.tile([128, 4, D], bf, tag="wo")

        # ---- phase 1: Q/K/V projections + RoPE, ts-major ----------------
        for cb in range(4):
            nc.sync.dma_start(wo_sb[:, cb], wo_d[:, cb])

        def rope_group(qkcb, ts):
            wt = wts[qkcb]
            ps = pmm.tile([128, 512], f32, tag="mm", name="psq")
            for k in range(KB):
                nc.tensor.matmul(ps[:], wt[:, k], xsb[:, k, ts * 512:(ts + 1) * 512],
                                 start=(k == 0), stop=(k == KB - 1))
            tsl = slice(ts * 512, (ts + 1) * 512)
            raw = tmp.tile([128, 512], bf, tag="raw")
            nc.scalar.copy(raw[:], ps[:])
            x1, x2 = raw[0:64, :], raw[64:128, :]
            t1 = rtmp.tile([64, 512], bf, tag="t1")
            t2 = rtmp.tile([64, 512], bf, tag="t2")
            nc.vector.tensor_tensor(t1[:], x2, sinv[:, tsl], mult)
            nc.vector.tensor_tensor(t2[:], x1, cosv[:, tsl], mult)
            nc.vector.tensor_tensor(qk[0:64, qkcb, tsl], t2[:], t1[:], sub)
            t3 = rtmp.tile([64, 512], bf, tag="t1")
            t4 = rtmp.tile([64, 512], bf, tag="t2")
            nc.vector.tensor_tensor(t3[:], x2, cosv64[:, tsl], mult)
            nc.vector.tensor_tensor(t4[:], x1, sinv0[:, tsl], mult)
            nc.vector.tensor_tensor(qk[64:128, qkcb, tsl], t4[:], t3[:], add)

        def v_group(tb):
            ps = pmm.tile([128, 512], f32, tag="mm", name="psv")
            for k in range(KB):
                nc.tensor.matmul(ps[:, 0:256], xsb[:, k, tb * 128:(tb + 1) * 128],
                                 wv[:, k], start=(k == 0), stop=(k == KB - 1))
            nc.scalar.copy(vsb[:, tb, 0:256], ps[:, 0:256])

        for ts in range(TSB):
            for qkcb in (4, 5, 0, 1, 2, 3):
                rope_group(qkcb, ts)
            for tb in range(ts * 4, ts * 4 + 4):
                v_group(tb)

        # ---- phase 2: attention, with o-projection interleaved ----------
        def attention(h, b, J, o2s):
            nk = 4 * (J + 1)
            qsl = slice(b * T + J * 512, b * T + (J + 1) * 512)
            e1 = epool.tile([128, QB, 512], bf, tag="e1", name="e1")
            e2 = epool.tile([128, QB, 512], bf, tag="e2", name="e2")
            for s, et in ((0, e1), (1, e2)):
                qblk = qk[:, 2 * h + s, qsl]
                for i in range(nk):
                    ksl = slice(b * T + i * 128, b * T + (i + 1) * 128)
                    ps = pmm.tile([128, 512], f32, tag="mm", name="pssc")
                    nc.tensor.matmul(ps[:], qk[:, 4 + s, ksl], qblk,
                                     start=True, stop=True)
                    nc.scalar.activation(et[:, i, :], ps[:],
                                         mybir.ActivationFunctionType.Exp)
                    if i >= 4 * J:      # diagonal sub-block masking
                        jl = i - 4 * J
                        qq = slice(jl * 128, (jl + 1) * 128)
                        nc.vector.tensor_tensor(et[:, i, qq],
                                                et[:, i, qq], tri[:], mult)
            drained = []
            for jl in range(4):     # phase A: AV matmuls + fast u-psum drain
                jg = 4 * J + jl
                qq = slice(jl * 128, (jl + 1) * 128)
                u1f = pav.tile([128, 512], f32, tag="u1", name="u1")
                u2f = pav.tile([128, 512], f32, tag="u2", name="u2")
                u1 = u1f[:, 0:257]
                u2 = u2f[:, 0:257]
                for i in range(jg + 1):
                    vt = vsb[:, b * QB + i, :]
                    nc.tensor.matmul(u1, e1[:, i, qq], vt,
                                     start=(i == 0), stop=(i == jg),
                                     skip_group_check=True)
                    nc.tensor.matmul(u2, e2[:, i, qq], vt,
                                     start=(i == 0), stop=(i == jg),
                                     skip_group_check=True)
                r1 = spool.tile([128, 1], f32, tag="r1", name=f"r1_{jl}")
                r2 = spool.tile([128, 1], f32, tag="r2", name=f"r2_{jl}")
                nc.vector.reciprocal(r1[:], u1[:, 256:257])
                nc.vector.reciprocal(r2[:], u2[:, 256:257])
                u1b = opool.tile([128, 256], bf, tag="u1b", name=f"u1b_{jl}")
                u2b = opool.tile([128, 256], bf, tag="u2b", name=f"u2b_{jl}")
                nc.vector.tensor_copy(u1b[:], u1[:, 0:256])
                nc.vector.tensor_copy(u2b[:], u2[:, 0:256])
                drained.append((jg, r1, r2, u1b, u2b))
            for jl, (jg, r1, r2, u1b, u2b) in enumerate(drained):
                # phase B: combine streams + GroupNorm over 256 channels
                nc.vector.tensor_scalar_mul(r2[:], r2[:], -lam)
                o1 = opool.tile([128, 256], bf, tag="o1", name=f"o1_{jl}")
                nc.vector.tensor_scalar_mul(o1[:], u1b[:], r1[:])
                nc.vector.scalar_tensor_tensor(o1[:], u2b[:], r2[:],
                                               o1[:], mult, add)
                st6 = spool.tile([128, 6], f32, tag="st6")
                mv = spool.tile([128, 2], f32, tag="mv")
                nc.vector.bn_stats(st6[:], o1[:])
                nc.vector.bn_aggr(mv[:], st6[:])
                sd = spool.tile([128, 1], f32, tag="sd")
                rstd = spool.tile([128, 1], f32, tag="rstd")
                nc.vector.tensor_scalar_add(sd[:], mv[:, 1:2], GN_EPS)
                nc.scalar.sqrt(sd[:], sd[:])
                nc.vector.reciprocal(rstd[:], sd[:])
                o2 = opool2.tile([128, 256], bf, tag="o2", name=f"o2_{jl}")
                nc.vector.tensor_scalar(o2[:], o1[:], mv[:, 0:1], rstd[:],
                                        sub, mult)
                o2s.append((h, jg, o2))

        def oproj_tb(tb):
                for np_ in range(2):
                    pso = [pop.tile([128, 512], f32, tag="op", name=f"pso{n}")
                           for n in range(2)]
                    for cb in range(4):
                        for ni, n in enumerate((2 * np_, 2 * np_ + 1)):
                            nc.tensor.matmul(pso[ni][:],
                                             osbT[:, cb, tb * 128:(tb + 1) * 128],
                                             wo_sb[:, cb, n * 512:(n + 1) * 512],
                                             start=(cb == 0), stop=(cb == 3),
                                             skip_group_check=True)
                    for ni, n in enumerate((2 * np_, 2 * np_ + 1)):
                        so = stage.tile([128, 512], bf, tag="so")
                        nc.scalar.copy(so[:], pso[ni][:])
                        nc.sync.dma_start(out_d[tb * 128:(tb + 1) * 128,
                                                n * 512:(n + 1) * 512], so[:])

        prev = None
        for b in range(B):
            for J in range(2):
                o2s = []
                for h in range(HPC):
                    attention(h, b, J, o2s)
                if prev is not None:
                    for tb in prev:
                        oproj_tb(tb)
                # transposes (q,ch)->(ch,q) into resident osbT
                for h2, jg, o2 in o2s:
                    for half in range(2):
                        pst = pop.tile([128, 512], bf, tag="op", name="pst")
                        nc.tensor.transpose(pst[:, 0:128],
                                            o2[:, half * 128:(half + 1) * 128],
                                            ident[:])
                        nc.vector.tensor_copy(
                            osbT[:, 2 * h2 + half,
                                 b * T + jg * 128:b * T + (jg + 1) * 128],
                            pst[:, 0:128])
                prev = [b * 8 + 4 * J + jl for jl in range(4)]
        for tb in prev:
            oproj_tb(tb)


# ------------------------- host side  ------------------------------------

_ROPE_PERM = np.concatenate([np.arange(0, DH, 2), np.arange(1, DH, 2)])


def _prep(x, Wq, Wk, Wv, Wo, lambda_q1, lambda_k1, lambda_q2, lambda_k2,
          gn_weight, gn_bias, pos):
    lam = float(np.exp(np.sum(lambda_q1 * lambda_k1))
                - np.exp(np.sum(lambda_q2 * lambda_k2)) + LAMBDA_INIT)
    scale = DH ** -0.5

    posf = pos.astype(np.float64)
    inv = 1.0 / (ROPE_BASE ** (np.arange(0, DH, 2, dtype=np.float32) / DH))
    freqs = (posf[:, None] * inv[None, :].astype(np.float64)).astype(np.float32)
    cosv = np.cos(freqs).T          # (64, T)
    sinv = np.sin(freqs).T
    tbl = np.concatenate([np.tile(cosv, (1, B)), np.tile(sinv, (1, B))],
                         axis=0).astype(bf16)          # (128, TOK) [cos;sin]
    tbl = np.ascontiguousarray(tbl)
    tbl2 = np.concatenate([np.tile(sinv, (1, B)), np.tile(cosv, (1, B))],
                          axis=0).astype(bf16)         # (128, TOK) [sin;cos]
    tbl2 = np.ascontiguousarray(tbl2)

    # x transposed: (D, B*T) -> (TSB, 128, KB, 512) ts-major contiguous
    xT = x.reshape(TOK, D).T.astype(np.float32)
    x3 = xT.reshape(KB, 128, TSB, 512).transpose(2, 1, 0, 3)
    xT3 = np.ascontiguousarray(x3).astype(bf16)

    # Wq: (H,2,DH,D), rope-permute DH, fold score scale
    Wq4 = (Wq.reshape(H, 2, DH, D)[:, :, _ROPE_PERM, :] * scale).astype(np.float32)
    Wk4 = Wk.reshape(KV, 2, DH, D)[:, :, _ROPE_PERM, :].astype(np.float32)
    Wv3 = Wv.reshape(KV, 2 * DH, D).astype(np.float32)

    s1 = 1.0 - LAMBDA_INIT
    Wo_f = (Wo * (gn_weight * s1)[None, :]).astype(np.float32)   # (D, 4096)
    bias_out = (gn_bias * s1).astype(np.float32) @ Wo.T.astype(np.float32)

    def to_sb(w2d, cols):           # (D, cols) -> (128, KB, cols) bf16
        return np.ascontiguousarray(
            w2d.reshape(KB, 128, cols).transpose(1, 0, 2)).astype(bf16)

    in_maps = []
    for c in range(NCORES):
        wqT = Wq4[2 * c:2 * c + 2].reshape(512, D).T          # (D, 512)
        wkT = Wk4[c // 2].reshape(256, D).T                   # (D, 256)
        wvT = Wv3[c // 2].T                                   # (D, 256)
        woT = Wo_f[:, 512 * c:512 * c + 512].T                # (512 ch, D out)
        wo3 = np.ascontiguousarray(
            woT.reshape(4, 128, D).transpose(1, 0, 2)).astype(bf16)
        wq_stack = np.stack([to_sb(wqT[:, cb * 128:(cb + 1) * 128], 128)
                             for cb in range(4)])             # (4,128,KB,128)
        wk_stack = np.stack([to_sb(wkT[:, cb * 128:(cb + 1) * 128], 128)
                             for cb in range(2)])             # (2,128,KB,128)
        in_maps.append({
            "xT": xT3,
            "wqT": wq_stack,
            "wkT": wk_stack,
            "wvT": to_sb(wvT, 256),
            "woT": wo3,
            "tbl": tbl,
            "tbl2": tbl2,
        })
    return lam, in_maps, bias_out


LAST_RESULT = None


def kernel(**inputs):
    global LAST_RESULT
    inputs = {k: np.asarray(v) for k, v in inputs.items()}
    lam, in_maps, bias_out = _prep(**inputs)
    nc = build_program(lam)
    res = run_bass_kernel_spmd(nc, in_maps, core_ids=list(range(NCORES)))
    LAST_RESULT = res
    out = np.zeros((TOK, D), np.float32)
    for c in range(NCORES):
        out += res.results[c]["out"].astype(np.float32)
    out += bias_out[None, :]
    return out.reshape(B, T, D).astype(np.float32)


if __name__ == "__main__":
    import reference
    ins = {k: np.asarray(v) for k, v in reference.setup_inputs().items()}
    got = kernel(**ins)
    exp = np.asarray(reference.reference(**ins))
    rel = np.linalg.norm(got - exp) / np.linalg.norm(exp)
    print("rel err:", rel)



# revision 37
# speedup vs baseline: 1.1439x; 1.1439x over previous
"""Trainium2 Bass kernel for nn_AttnLayerV3 (differential attention layer).

Tensor-parallel over heads across 8 NeuronCores:
  - core c owns q-heads {2c, 2c+1} and kv-head c//2 (duplicated per core pair)
  - per-core: Q/K/V projections, RoPE, two-stream causal softmax attention
    (computed transposed: scores (k, q) so attention weights are directly the
    lhsT of the AV matmul), per-head GroupNorm, partial output projection
  - host: shards/permutes weights, gathers the 8 partial outputs and sums.

Engine assignment (v2):
  - PE: all matmuls (proj / scores / AV / o-proj / transposes)
  - Act: exp, PSUM->SBUF drains (Copy), GN rstd via ln/exp (exp/ln/copy all
    live in the natural_log_exp activation table set -> zero table reloads)
  - Pool (gpsimd): SBUF-only work (RoPE swapped-half build, diag masking);
    gpsimd cannot touch PSUM on TRN2
  - DVE: RoPE combines (3 full-width ops), softmax normalize, GroupNorm stats

Scheduling: exact-causal score subranges; projection/o-proj matmuls are
software-pipelined (generator "filler" units) into the Act-bound score
phases so the PE never idles waiting for exp drains.

All matmuls run in bf16 with fp32 PSUM accumulation.
"""

import numpy as np
import ml_dtypes

import concourse.bass as bass
import concourse.bacc as bacc
import concourse.tile as tile
import concourse.mybir as mybir
from concourse.bass_utils import run_bass_kernel_spmd
from concourse.masks import make_identity, make_upper_triangular

bf16 = ml_dtypes.bfloat16

B, T, D = 2, 1024, 2048
H, KV, DH = 16, 4, 128
NCORES = 8
HPC = H // NCORES          # q heads per core = 2
TOK = B * T                # 2048
LAMBDA_INIT = 0.8 - 0.6 * float(np.exp(-0.3 * 1))
GN_EPS = 1e-5
ROPE_BASE = 10000.0

KB = 16                    # contraction blocks of 128 over D
TSB = TOK // 512           # 4 token superblocks of 512
QB = T // 128              # 8 q/k blocks of 128 per batch
NTB = TOK // 128           # 16 token blocks of 128


def _shape_act_tables(arch: str):
    """Constrain the act-table selection so Exp/Ln resolve to the single
    table set that holds both (natural_log_exp_and_others). Only the
    (cached) selection metadata is narrowed; emitted set ids still index
    the unmodified act_info.json, so the loaded tables are correct. Net
    effect: one act-table load for the whole kernel instead of one per
    Exp<->Ln transition."""
    from concourse.hw_specs import get_activation_tables
    tabs = get_activation_tables(arch)
    keep = "natural_log_exp_and_others"
    if keep not in tabs:
        return
    drop = {mybir.ActivationFunctionType.Exp, mybir.ActivationFunctionType.Ln}
    for name, funcs in tabs.items():
        if name != keep:
            funcs -= drop


def build_program(lam: float):
    f32 = mybir.dt.float32
    bf = mybir.dt.bfloat16
    nc = bacc.Bacc("TRN2", target_bir_lowering=False, debug=False,
                   num_devices=NCORES)
    _shape_act_tables(nc.m.arch)

    xT_d = nc.dram_tensor("xT", (TSB, 128, KB, 512), bf, kind="ExternalInput").ap()
    wq_d = nc.dram_tensor("wqT", (4, 128, KB, 128), bf, kind="ExternalInput").ap()
    wk_d = nc.dram_tensor("wkh", (128, KB, 128), bf, kind="ExternalInput").ap()
    wv_d = nc.dram_tensor("wvh", (128, KB, 128), bf, kind="ExternalInput").ap()
    wo_d = nc.dram_tensor("woT", (128, 4, D), bf, kind="ExternalInput").ap()
    tbl_d = nc.dram_tensor("tbl", (128, TOK), bf, kind="ExternalInput").ap()
    tb2_d = nc.dram_tensor("tbl2", (128, TOK), bf, kind="ExternalInput").ap()
    out_d = nc.dram_tensor("out", (TOK, D), bf, kind="ExternalOutput").ap()

    with tile.TileContext(nc) as tc:
        _body(tc, xT_d, wq_d, wk_d, wv_d, wo_d, tbl_d, tb2_d, out_d, lam)
    nc.compile()
    return nc


def _body(tc, xT_d, wq_d, wk_d, wv_d, wo_d, tbl_d, tb2_d, out_d, lam):
    nc = tc.nc
    f32 = mybir.dt.float32
    bf = mybir.dt.bfloat16
    mult = mybir.AluOpType.mult
    sub = mybir.AluOpType.subtract
    add = mybir.AluOpType.add
    Exp = mybir.ActivationFunctionType.Exp
    Ln = mybir.ActivationFunctionType.Ln

    import contextlib
    ctx = contextlib.ExitStack()
    with ctx:
        # ---- pools -------------------------------------------------------
        big = ctx.enter_context(tc.tile_pool(name="big", bufs=1))
        xpool = ctx.enter_context(tc.tile_pool(name="xp", bufs=2))
        wpool = ctx.enter_context(tc.tile_pool(name="wq", bufs=6))
        tmp = ctx.enter_context(tc.tile_pool(name="tmp", bufs=2))
        rtmp = ctx.enter_context(tc.tile_pool(name="rtmp", bufs=2))
        opool = ctx.enter_context(tc.tile_pool(name="o", bufs=5))
        opool2 = ctx.enter_context(tc.tile_pool(name="o2", bufs=8))
        spool = ctx.enter_context(tc.tile_pool(name="s", bufs=5))
        epool = ctx.enter_context(tc.tile_pool(name="e", bufs=2))
        stage = ctx.enter_context(tc.tile_pool(name="stage", bufs=3))
        dram = ctx.enter_context(tc.tile_pool(name="dram", bufs=2, space="DRAM"))
        pmm = ctx.enter_context(tc.tile_pool(name="pmm", bufs=2, space="PSUM"))
        pav = ctx.enter_context(tc.tile_pool(name="pav", bufs=2, space="PSUM"))
        pop = ctx.enter_context(tc.tile_pool(name="pop", bufs=2, space="PSUM"))
        # PSUM banks: pmm 2 (scores/phase-1 proj) + pav 4 (u1,u2 x2)
        #           + pop 2 (filler proj, o-proj, transposes) = 8

        # ---- constants ---------------------------------------------------
        ident = big.tile([128, 128], bf, tag="ident")
        make_identity(nc, ident[:])
        tri = big.tile([128, 128], bf, tag="tri")
        make_upper_triangular(nc, tri[:], val=1.0, diag=True)

        # DMA priority order: each tensor queued just ahead of its first use.
        wts = {}
        xts = [None] * TSB

        def load_x(ts, chunked=False):
            xt = xpool.tile([128, KB, 512], bf, tag="xsb", name=f"x{ts}")
            if chunked:
                for kc in range(4):
                    nc.sync.dma_start(xt[:, kc * 4:(kc + 1) * 4, :],
                                      xT_d[ts][:, kc * 4:(kc + 1) * 4])
            else:
                nc.sync.dma_start(xt[:], xT_d[ts])
            xts[ts] = xt

        # wkh/x0 interleaved in k-chunks so matmul k=0 starts ~1.5us in
        wt0 = wpool.tile([128, KB, 128], bf, tag="w", name="wtkh")
        xt0 = xpool.tile([128, KB, 512], bf, tag="xsb", name="x0")
        for kc in range(4):
            nc.sync.dma_start(wt0[:, kc * 4:(kc + 1) * 4, :],
                              wk_d[:, kc * 4:(kc + 1) * 4])
            nc.sync.dma_start(xt0[:, kc * 4:(kc + 1) * 4, :],
                              xT_d[0][:, kc * 4:(kc + 1) * 4])
        wts[4] = wt0
        xts[0] = xt0
        for cb in range(4):
            wt = wpool.tile([128, KB, 128], bf, tag="w", name=f"wtq{cb}")
            wts[cb] = wt
        nc.sync.dma_start(wts[0][:], wq_d[0])
        nc.sync.dma_start(wts[1][:], wq_d[1])
        tblA = big.tile([128, TOK], bf, tag="tblA")      # [cos; cos]
        nc.sync.dma_start(tblA[:], tbl_d)
        tblB = big.tile([128, TOK], bf, tag="tblB")      # [-sin; sin]
        nc.sync.dma_start(tblB[:], tb2_d)
        nc.sync.dma_start(wts[2][:], wq_d[2])
        nc.sync.dma_start(wts[3][:], wq_d[3])
        wv = big.tile([128, KB, 128], bf, tag="wv")
        nc.sync.dma_start(wv[:], wv_d)
        load_x(1)
        wo_sb = big.tile([128, 4, D], bf, tag="wo")
        for cb in range(4):
            nc.sync.dma_start(wo_sb[:, cb], wo_d[:, cb])

        # q streams (4 blocks: h0s0 h0s1 h1s0 h1s1) then k streams (2 blocks)
        epst = big.tile([128, 1], f32, tag="epst")
        nc.vector.memset(epst[:], GN_EPS)
        qk = big.tile([128, 6, TOK], bf, tag="qk")
        # v blocks + two norm columns: col 256 = 1 (stream-1 softmax sum),
        # col 257 = -1/lam (stream-2 sum pre-scaled so its reciprocal is
        # directly the -lam/z2 combine factor)
        vsb = big.tile([128, NTB, 258], bf, tag="vsb")
        nc.vector.memset(vsb[:, :, 256:257], 1.0)
        nc.vector.memset(vsb[:, :, 257:258], -1.0 / lam)
        osbT = big.tile([128, 4, TOK], bf, tag="osbT")   # o transposed, ch-major

        # ---- unit generators (yield once per PE matmul) ------------------
        def rope_group(qkcb, ts, pool, ptag):
            wt = wts[qkcb]
            xt = xts[ts]
            tsl = slice(ts * 512, (ts + 1) * 512)
            ps = pool.tile([128, 512], f32, tag=ptag, name="psq")
            for k in range(KB):
                nc.tensor.matmul(ps[:], wt[:, k], xt[:, k, :],
                                 start=(k == 0), stop=(k == KB - 1),
                                 skip_group_check=True)
                yield
            raw = tmp.tile([128, 512], bf, tag="raw")    # [x1; x2]
            raw2 = tmp.tile([128, 512], bf, tag="raw2")  # [x2; x1]
            nc.scalar.copy(raw[:], ps[:])
            nc.vector.tensor_copy(raw2[0:64, :], raw[64:128, :])
            nc.vector.tensor_copy(raw2[64:128, :], raw[0:64, :])
            p_ = rtmp.tile([128, 512], bf, tag="p")
            q_ = rtmp.tile([128, 512], bf, tag="q")
            # tblA = [cos; cos], tblB = [-sin; sin] (host-baked signs)
            nc.vector.tensor_tensor(p_[:], raw[:], tblA[:, tsl], mult)
            nc.vector.tensor_tensor(q_[:], raw2[:], tblB[:, tsl], mult)
            nc.vector.tensor_tensor(qk[:, qkcb, tsl], p_[:], q_[:], add)

        def v_group(tb, pool, ptag):
            # computes only this core's parity-half of the v channels; the
            # pair AllGather fills vsb[:, :, 0:256] on both cores
            xt = xts[tb // 4]
            to = (tb % 4) * 128
            ps = pool.tile([128, 512], f32, tag=ptag, name="psv")
            for k in range(KB):
                nc.tensor.matmul(ps[:, 0:128], xt[:, k, to:to + 128], wv[:, k],
                                 start=(k == 0), stop=(k == KB - 1),
                                 skip_group_check=True)
                yield
            nc.scalar.copy(vsb[:, tb, 0:128], ps[:, 0:128])

        def do_collective(b):
            # pair {2c, 2c+1} exchange: my K stream + my half of V channels
            bsl = slice(b * T, (b + 1) * T)
            tbs = slice(b * 8, b * 8 + 8)
            ccin = dram.tile([128, 2048], bf, tag="ccin", name=f"ccin{b}")
            ccout = dram.tile([2, 128, 2048], bf, tag="ccout", name=f"ccout{b}")
            nc.gpsimd.dma_start(ccin[:, 0:1024], qk[:, 4, bsl])
            nc.gpsimd.dma_start(ccin[:, 1024:2048], vsb[:, tbs, 0:128])
            nc.gpsimd.collective_compute(
                "AllGather", mybir.AluOpType.bypass,
                replica_groups=[[0, 1], [2, 3], [4, 5], [6, 7]],
                ins=[ccin[:].opt()], outs=[ccout[:].opt()])
            nc.gpsimd.dma_start(qk[:, 4, bsl], ccout[0, :, 0:1024])
            nc.gpsimd.dma_start(qk[:, 5, bsl], ccout[1, :, 0:1024])
            nc.gpsimd.dma_start(vsb[:, tbs, 0:128], ccout[0, :, 1024:2048])
            nc.gpsimd.dma_start(vsb[:, tbs, 128:256], ccout[1, :, 1024:2048])

        def oproj_tb(tb):
            for np_ in range(2):
                pso = [pop.tile([128, 512], f32, tag="op", name=f"pso{n}")
                       for n in range(2)]
                for cb in range(4):
                    for ni, n in enumerate((2 * np_, 2 * np_ + 1)):
                        nc.tensor.matmul(pso[ni][:],
                                         osbT[:, cb, tb * 128:(tb + 1) * 128],
                                         wo_sb[:, cb, n * 512:(n + 1) * 512],
                                         start=(cb == 0), stop=(cb == 3),
                                         skip_group_check=True)
                        yield
                for ni, n in enumerate((2 * np_, 2 * np_ + 1)):
                    so = stage.tile([128, 512], bf, tag="so")
                    nc.scalar.copy(so[:], pso[ni][:])
                    nc.sync.dma_start(out_d[tb * 128:(tb + 1) * 128,
                                            n * 512:(n + 1) * 512], so[:])

        # ---- phase 1: projections; K/V halves feed pair AllGathers -------
        for ts in range(4):
            ropes = (4, 0, 1, 2, 3) if ts < 3 else (4,)
            for qkcb in ropes:
                for _ in rope_group(qkcb, ts, pmm, "mm"):
                    pass
            for tb in range(ts * 4, ts * 4 + 4):
                for _ in v_group(tb, pmm, "mm"):
                    pass
            if ts == 1:
                do_collective(0)    # b0 K/V exchange overlaps ts2/ts3 proj
            if ts < 2:
                load_x(ts + 2)      # recycles xt[ts] buffer: all reads done
        do_collective(1)

        # ---- phase 2: attention with pipelined filler --------------------
        def scores_part(h, b, J, fill):
            nk = 4 * (J + 1)
            qsl = slice(b * T + J * 512, b * T + (J + 1) * 512)
            e1 = epool.tile([128, nk, 512], bf, tag="e1", name="e1")
            e2 = epool.tile([128, nk, 512], bf, tag="e2", name="e2")
            for s, et in ((0, e1), (1, e2)):
                qblk = qk[:, 2 * h + s, qsl]
                for i in range(nk):
                    lo = max(0, i - 4 * J) * 128
                    ksl = slice(b * T + i * 128, b * T + (i + 1) * 128)
                    ps = pmm.tile([128, 512], f32, tag="mm", name="pssc")
                    nc.tensor.matmul(ps[:, lo:512], qk[:, 4 + s, ksl],
                                     qblk[:, lo:512], start=True, stop=True,
                                     skip_group_check=True)
                    next(fill, None)
                    if lo == 0:
                        next(fill, None)
                    nc.scalar.activation(et[:, i, lo:512], ps[:, lo:512], Exp)
                    if i >= 4 * J:      # diagonal sub-block masking
                        nc.vector.tensor_tensor(et[:, i, lo:lo + 128],
                                                et[:, i, lo:lo + 128],
                                                tri[:], mult)
            return e1, e2

        def gn_tail(h, jg, o1, o2s):
            # GroupNorm over 256 channels -> o2
            st6 = spool.tile([128, 6], f32, tag="st6")
            mv = spool.tile([128, 2], f32, tag="mv")
            nc.vector.bn_stats(st6[:], o1[:])
            nc.vector.bn_aggr(mv[:], st6[:])
            lnv = spool.tile([128, 1], f32, tag="lnv")
            rstd = spool.tile([128, 1], f32, tag="rstd")
            # rstd = (var+eps)^-0.5 via ln/exp (same act table as Exp)
            nc.scalar.activation(lnv[:], mv[:, 1:2], Ln, bias=epst[:])
            nc.scalar.activation(rstd[:], lnv[:], Exp, scale=-0.5)
            o2 = opool2.tile([128, 256], bf, tag="o2", name="o2")
            nc.vector.tensor_scalar(o2[:], o1[:], mv[:, 0:1], rstd[:],
                                    sub, mult)
            o2s.append((h, jg, o2))

        def av_part(h, b, J, e1, e2, o2s, fill, direct=False):
            drained = []
            for jl in range(4):     # phase A: AV matmuls + fast u-psum drain
                jg = 4 * J + jl
                qq = slice(jl * 128, (jl + 1) * 128)
                u1f = pav.tile([128, 512], f32, tag="u1", name="u1")
                u2f = pav.tile([128, 512], f32, tag="u2", name="u2")
                u1 = u1f[:, 0:258]
                u2 = u2f[:, 0:258]
                for i in range(jg + 1):
                    vt = vsb[:, b * QB + i, :]
                    nc.tensor.matmul(u1, e1[:, i, qq], vt,
                                     start=(i == 0), stop=(i == jg),
                                     skip_group_check=True)
                    nc.tensor.matmul(u2, e2[:, i, qq], vt,
                                     start=(i == 0), stop=(i == jg),
                                     skip_group_check=True)
                next(fill, None)
                next(fill, None)
                r1 = spool.tile([128, 1], f32, tag="r1", name=f"r1_{jl}")
                r2 = spool.tile([128, 1], f32, tag="r2", name=f"r2_{jl}")
                nc.vector.reciprocal(r1[:], u1[:, 256:257])
                nc.vector.reciprocal(r2[:], u2[:, 257:258])   # = -lam/z2
                if direct:
                    # combine straight from PSUM: shorter chain to the
                    # transposes (used for the final group's tail)
                    o1 = opool.tile([128, 256], bf, tag="o1", name=f"o1_{jl}")
                    nc.vector.tensor_scalar_mul(o1[:], u1[:, 0:256], r1[:])
                    nc.vector.scalar_tensor_tensor(o1[:], u2[:, 0:256], r2[:],
                                                   o1[:], mult, add)
                    gn_tail(h, jg, o1, o2s)
                    continue
                u1b = opool.tile([128, 256], bf, tag="u1b", name=f"u1b_{jl}")
                u2b = opool.tile([128, 256], bf, tag="u2b", name=f"u2b_{jl}")
                nc.scalar.copy(u1b[:], u1[:, 0:256])      # Act
                nc.vector.tensor_copy(u2b[:], u2[:, 0:256])  # DVE
                drained.append((jg, r1, r2, u1b, u2b))
            for jl, (jg, r1, r2, u1b, u2b) in enumerate(drained):
                # phase B: combine streams + GroupNorm over 256 channels
                o1 = opool.tile([128, 256], bf, tag="o1", name=f"o1_{jl}")
                nc.vector.tensor_scalar_mul(o1[:], u1b[:], r1[:])
                nc.vector.scalar_tensor_tensor(o1[:], u2b[:], r2[:],
                                               o1[:], mult, add)
                gn_tail(h, jg, o1, o2s)

        def unit_chain(units):
            for u in units:
                yield from u

        def transpose_block(b, h2, jg, o2, half):
            pst = pop.tile([128, 512], bf, tag="op", name="pst")
            nc.tensor.transpose(pst[:, 0:128],
                                o2[:, half * 128:(half + 1) * 128],
                                ident[:])
            nc.vector.tensor_copy(
                osbT[:, 2 * h2 + half,
                     b * T + jg * 128:b * T + (jg + 1) * 128],
                pst[:, 0:128])

        def do_group(b, J, units, tail_tbs=None):
            fill = unit_chain(units)
            o2s = []
            for h in range(HPC):
                e1, e2 = scores_part(h, b, J, fill)
                av_part(h, b, J, e1, e2, o2s, fill,
                        direct=(tail_tbs is not None))
            for _ in fill:          # flush remaining filler work
                pass
            # transposes (q,ch)->(ch,q) into resident osbT
            if tail_tbs is None:
                for h2, jg, o2 in o2s:
                    for half in range(2):
                        transpose_block(b, h2, jg, o2, half)
            else:
                # last group: per-q-block, both heads, then that token
                # block's o-projection immediately (shrinks the end tail)
                byjl = {(h2, jg): o2 for h2, jg, o2 in o2s}
                for jl in range(4):
                    jg = 4 * J + jl
                    for h2 in range(HPC):
                        for half in range(2):
                            transpose_block(b, h2, jg, byjl[(h2, jg)], half)
                    for _ in oproj_tb(tail_tbs[jl]):
                        pass

        # q-ropes for ts3: two inline (slack for the b0 collective to land),
        # two as filler inside the first attention group
        for _ in rope_group(0, 3, pmm, "mm"):
            pass
        for _ in rope_group(1, 3, pmm, "mm"):
            pass
        do_group(0, 0, [rope_group(c, 3, pop, "op") for c in (2, 3)])
        do_group(0, 1, [oproj_tb(tb) for tb in (0, 1, 2, 3)])
        do_group(1, 0, [oproj_tb(tb) for tb in (4, 5, 6, 7)])
        do_group(1, 1, [oproj_tb(tb) for tb in (8, 9, 10, 11)],
                 tail_tbs=(12, 13, 14, 15))


# ------------------------- host side  ------------------------------------

_ROPE_PERM = np.concatenate([np.arange(0, DH, 2), np.arange(1, DH, 2)])


def _prep(x, Wq, Wk, Wv, Wo, lambda_q1, lambda_k1, lambda_q2, lambda_k2,
          gn_weight, gn_bias, pos):
    lam = float(np.exp(np.sum(lambda_q1 * lambda_k1))
                - np.exp(np.sum(lambda_q2 * lambda_k2)) + LAMBDA_INIT)
    if abs(lam) < 1e-6:             # keep the baked -1/lam column finite
        lam = 1e-6 if lam >= 0 else -1e-6
    scale = DH ** -0.5

    posf = pos.astype(np.float64)
    inv = 1.0 / (ROPE_BASE ** (np.arange(0, DH, 2, dtype=np.float32) / DH))
    freqs = (posf[:, None] * inv[None, :].astype(np.float64)).astype(np.float32)
    cosv = np.cos(freqs).T          # (64, T)
    sinv = np.sin(freqs).T
    cost = np.tile(cosv, (1, B))
    sint = np.tile(sinv, (1, B))
    tblA = np.ascontiguousarray(
        np.concatenate([cost, cost], axis=0)).astype(bf16)   # [cos; cos]
    tblB = np.ascontiguousarray(
        np.concatenate([-sint, sint], axis=0)).astype(bf16)  # [-sin; sin]

    # x transposed: (D, B*T) -> (TSB, 128, KB, 512) ts-major contiguous
    xT = x.reshape(TOK, D).T.astype(np.float32)
    x3 = xT.reshape(KB, 128, TSB, 512).transpose(2, 1, 0, 3)
    xT3 = np.ascontiguousarray(x3).astype(bf16)

    # Wq: (H,2,DH,D), rope-permute DH, fold score scale
    Wq4 = (Wq.reshape(H, 2, DH, D)[:, :, _ROPE_PERM, :] * scale).astype(np.float32)
    Wk4 = Wk.reshape(KV, 2, DH, D)[:, :, _ROPE_PERM, :].astype(np.float32)
    Wv3 = Wv.reshape(KV, 2 * DH, D).astype(np.float32)

    s1 = 1.0 - LAMBDA_INIT
    Wo_f = (Wo * (gn_weight * s1)[None, :]).astype(np.float32)   # (D, 4096)
    bias_out = (gn_bias * s1).astype(np.float32) @ Wo.T.astype(np.float32)

    def to_sb(w2d, cols):           # (D, cols) -> (128, KB, cols) bf16
        return np.ascontiguousarray(
            w2d.reshape(KB, 128, cols).transpose(1, 0, 2)).astype(bf16)

    in_maps = []
    for c in range(NCORES):
        p = c % 2          # parity: which K stream / V half this core computes
        wqT = Wq4[2 * c:2 * c + 2].reshape(512, D).T          # (D, 512)
        wkhT = Wk4[c // 2, p].T                               # (D, 128)
        wvhT = Wv3[c // 2].T[:, p * 128:(p + 1) * 128]        # (D, 128)
        woT = Wo_f[:, 512 * c:512 * c + 512].T                # (512 ch, D out)
        wo3 = np.ascontiguousarray(
            woT.reshape(4, 128, D).transpose(1, 0, 2)).astype(bf16)
        wq_stack = np.stack([to_sb(wqT[:, cb * 128:(cb + 1) * 128], 128)
                             for cb in range(4)])             # (4,128,KB,128)
        in_maps.append({
            "xT": xT3,
            "wqT": wq_stack,
            "wkh": to_sb(wkhT, 128),
            "wvh": to_sb(wvhT, 128),
            "woT": wo3,
            "tbl": tblA,
            "tbl2": tblB,
        })
    return lam, in_maps, bias_out


LAST_RESULT = None


def kernel(**inputs):
    global LAST_RESULT
    inputs = {k: np.asarray(v) for k, v in inputs.items()}
    lam, in_maps, bias_out = _prep(**inputs)
    nc = build_program(lam)
    res = run_bass_kernel_spmd(nc, in_maps, core_ids=list(range(NCORES)))
    LAST_RESULT = res
    out = np.zeros((TOK, D), np.float32)
    for c in range(NCORES):
        out += res.results[c]["out"].astype(np.float32)
    out += bias_out[None, :]
    return out.reshape(B, T, D).astype(np.float32)


if __name__ == "__main__":
    import reference
    ins = {k: np.asarray(v) for k, v in reference.setup_inputs().items()}
    got = kernel(**ins)
    exp = np.asarray(reference.reference(**ins))
    rel = np.linalg.norm(got - exp) / np.linalg.norm(exp)
    print("rel err:", rel)


# revision 38
# speedup vs baseline: 1.2154x; 1.0625x over previous
"""Trainium2 Bass kernel for nn_AttnLayerV3 (differential attention layer).

Tensor-parallel over heads across 8 NeuronCores:
  - core c owns q-heads {2c, 2c+1} and kv-head c//2 (duplicated per core pair)
  - per-core: Q/K/V projections, RoPE, two-stream causal softmax attention
    (computed transposed: scores (k, q) so attention weights are directly the
    lhsT of the AV matmul), per-head GroupNorm, partial output projection
  - host: shards/permutes weights, gathers the 8 partial outputs and sums.

Engine assignment (v2):
  - PE: all matmuls (proj / scores / AV / o-proj / transposes)
  - Act: exp, PSUM->SBUF drains (Copy), GN rstd via ln/exp (exp/ln/copy all
    live in the natural_log_exp activation table set -> zero table reloads)
  - Pool (gpsimd): SBUF-only work (RoPE swapped-half build, diag masking);
    gpsimd cannot touch PSUM on TRN2
  - DVE: RoPE combines (3 full-width ops), softmax normalize, GroupNorm stats

Scheduling: exact-causal score subranges; projection/o-proj matmuls are
software-pipelined (generator "filler" units) into the Act-bound score
phases so the PE never idles waiting for exp drains.

All matmuls run in bf16 with fp32 PSUM accumulation.
"""

import numpy as np
import ml_dtypes

import concourse.bass as bass
import concourse.bacc as bacc
import concourse.tile as tile
import concourse.mybir as mybir
from concourse.bass_utils import run_bass_kernel_spmd
from concourse.masks import make_identity, make_upper_triangular

bf16 = ml_dtypes.bfloat16

B, T, D = 2, 1024, 2048
H, KV, DH = 16, 4, 128
NCORES = 8
HPC = H // NCORES          # q heads per core = 2
TOK = B * T                # 2048
LAMBDA_INIT = 0.8 - 0.6 * float(np.exp(-0.3 * 1))
GN_EPS = 1e-5
ROPE_BASE = 10000.0

KB = 16                    # contraction blocks of 128 over D
TSB = TOK // 512           # 4 token superblocks of 512
QB = T // 128              # 8 q/k blocks of 128 per batch
NTB = TOK // 128           # 16 token blocks of 128


def _shape_act_tables(arch: str):
    """Constrain the act-table selection so Exp/Ln resolve to the single
    table set that holds both (natural_log_exp_and_others). Only the
    (cached) selection metadata is narrowed; emitted set ids still index
    the unmodified act_info.json, so the loaded tables are correct. Net
    effect: one act-table load for the whole kernel instead of one per
    Exp<->Ln transition."""
    from concourse.hw_specs import get_activation_tables
    tabs = get_activation_tables(arch)
    keep = "natural_log_exp_and_others"
    if keep not in tabs:
        return
    drop = {mybir.ActivationFunctionType.Exp, mybir.ActivationFunctionType.Ln}
    for name, funcs in tabs.items():
        if name != keep:
            funcs -= drop


def build_program(lam: float):
    f32 = mybir.dt.float32
    bf = mybir.dt.bfloat16
    nc = bacc.Bacc("TRN2", target_bir_lowering=False, debug=False,
                   num_devices=NCORES)
    _shape_act_tables(nc.m.arch)

    xT_d = nc.dram_tensor("xT", (TSB, 128, KB, 512), bf, kind="ExternalInput").ap()
    wq_d = nc.dram_tensor("wqT", (4, 128, KB, 128), bf, kind="ExternalInput").ap()
    wk_d = nc.dram_tensor("wkh", (128, KB, 128), bf, kind="ExternalInput").ap()
    wv_d = nc.dram_tensor("wvh", (128, KB, 128), bf, kind="ExternalInput").ap()
    wo_d = nc.dram_tensor("woT", (128, 4, D), bf, kind="ExternalInput").ap()
    tbl_d = nc.dram_tensor("tbl", (128, TOK), bf, kind="ExternalInput").ap()
    tb2_d = nc.dram_tensor("tbl2", (128, TOK), bf, kind="ExternalInput").ap()
    out_d = nc.dram_tensor("out", (TOK, D), bf, kind="ExternalOutput").ap()

    with tile.TileContext(nc) as tc:
        _body(tc, xT_d, wq_d, wk_d, wv_d, wo_d, tbl_d, tb2_d, out_d, lam)
    nc.compile()
    return nc


def _body(tc, xT_d, wq_d, wk_d, wv_d, wo_d, tbl_d, tb2_d, out_d, lam):
    nc = tc.nc
    f32 = mybir.dt.float32
    bf = mybir.dt.bfloat16
    mult = mybir.AluOpType.mult
    sub = mybir.AluOpType.subtract
    add = mybir.AluOpType.add
    Exp = mybir.ActivationFunctionType.Exp
    Ln = mybir.ActivationFunctionType.Ln

    import contextlib
    ctx = contextlib.ExitStack()
    with ctx:
        # ---- pools -------------------------------------------------------
        big = ctx.enter_context(tc.tile_pool(name="big", bufs=1))
        xpool = ctx.enter_context(tc.tile_pool(name="xp", bufs=2))
        wpool = ctx.enter_context(tc.tile_pool(name="wq", bufs=6))
        tmp = ctx.enter_context(tc.tile_pool(name="tmp", bufs=2))
        rtmp = ctx.enter_context(tc.tile_pool(name="rtmp", bufs=2))
        opool = ctx.enter_context(tc.tile_pool(name="o", bufs=5))
        opool2 = ctx.enter_context(tc.tile_pool(name="o2", bufs=8))
        spool = ctx.enter_context(tc.tile_pool(name="s", bufs=5))
        epool = ctx.enter_context(tc.tile_pool(name="e", bufs=2))
        stage = ctx.enter_context(tc.tile_pool(name="stage", bufs=3))
        dram = ctx.enter_context(tc.tile_pool(name="dram", bufs=2, space="DRAM"))
        pmm = ctx.enter_context(tc.tile_pool(name="pmm", bufs=2, space="PSUM"))
        pav = ctx.enter_context(tc.tile_pool(name="pav", bufs=2, space="PSUM"))
        pop = ctx.enter_context(tc.tile_pool(name="pop", bufs=2, space="PSUM"))
        # PSUM banks: pmm 2 (scores/phase-1 proj) + pav 4 (u1,u2 x2)
        #           + pop 2 (filler proj, o-proj, transposes) = 8

        # ---- constants ---------------------------------------------------
        ident = big.tile([128, 128], bf, tag="ident")
        make_identity(nc, ident[:])
        tri = big.tile([128, 128], bf, tag="tri")
        make_upper_triangular(nc, tri[:], val=1.0, diag=True)

        # DMA priority order: each tensor queued just ahead of its first use.
        wts = {}
        xts = [None] * TSB

        def load_x(ts, chunked=False):
            xt = xpool.tile([128, KB, 512], bf, tag="xsb", name=f"x{ts}")
            if chunked:
                for kc in range(4):
                    nc.sync.dma_start(xt[:, kc * 4:(kc + 1) * 4, :],
                                      xT_d[ts][:, kc * 4:(kc + 1) * 4])
            else:
                nc.sync.dma_start(xt[:], xT_d[ts])
            xts[ts] = xt

        # wkh/x0 interleaved in k-chunks so matmul k=0 starts ~1.5us in
        wt0 = wpool.tile([128, KB, 128], bf, tag="w", name="wtkh")
        xt0 = xpool.tile([128, KB, 512], bf, tag="xsb", name="x0")
        for kc in range(4):
            nc.sync.dma_start(wt0[:, kc * 4:(kc + 1) * 4, :],
                              wk_d[:, kc * 4:(kc + 1) * 4])
            nc.sync.dma_start(xt0[:, kc * 4:(kc + 1) * 4, :],
                              xT_d[0][:, kc * 4:(kc + 1) * 4])
        wts[4] = wt0
        xts[0] = xt0
        for cb in range(4):
            wt = wpool.tile([128, KB, 128], bf, tag="w", name=f"wtq{cb}")
            wts[cb] = wt
        nc.sync.dma_start(wts[0][:], wq_d[0])
        nc.sync.dma_start(wts[1][:], wq_d[1])
        tblA = big.tile([128, TOK], bf, tag="tblA")      # [cos; cos]
        nc.sync.dma_start(tblA[:], tbl_d)
        tblB = big.tile([128, TOK], bf, tag="tblB")      # [-sin; sin]
        nc.sync.dma_start(tblB[:], tb2_d)
        nc.sync.dma_start(wts[2][:], wq_d[2])
        nc.sync.dma_start(wts[3][:], wq_d[3])
        wv = big.tile([128, KB, 128], bf, tag="wv")
        nc.sync.dma_start(wv[:], wv_d)
        load_x(1)
        wo_sb = big.tile([128, 4, D], bf, tag="wo")
        for cb in range(4):
            nc.sync.dma_start(wo_sb[:, cb], wo_d[:, cb])

        # q streams (4 blocks: h0s0 h0s1 h1s0 h1s1) then k streams (2 blocks)
        epst = big.tile([128, 1], f32, tag="epst")
        nc.vector.memset(epst[:], GN_EPS)
        qk = big.tile([128, 6, TOK], bf, tag="qk")
        # v blocks + two norm columns: col 256 = 1 (stream-1 softmax sum),
        # col 257 = -1/lam (stream-2 sum pre-scaled so its reciprocal is
        # directly the -lam/z2 combine factor)
        vsb = big.tile([128, NTB, 258], bf, tag="vsb")
        nc.vector.memset(vsb[:, :, 256:257], 1.0)
        nc.vector.memset(vsb[:, :, 257:258], -1.0 / lam)
        osbT = big.tile([128, 4, TOK], bf, tag="osbT")   # o transposed, ch-major

        # ---- unit generators (yield once per PE matmul) ------------------
        def rope_group(qkcb, ts, pool, ptag):
            wt = wts[qkcb]
            xt = xts[ts]
            tsl = slice(ts * 512, (ts + 1) * 512)
            ps = pool.tile([128, 512], f32, tag=ptag, name="psq")
            for k in range(KB):
                nc.tensor.matmul(ps[:], wt[:, k], xt[:, k, :],
                                 start=(k == 0), stop=(k == KB - 1),
                                 skip_group_check=True)
                yield
            raw = tmp.tile([128, 512], bf, tag="raw")    # [x1; x2]
            raw2 = tmp.tile([128, 512], bf, tag="raw2")  # [x2; x1]
            nc.scalar.copy(raw[:], ps[:])
            nc.vector.tensor_copy(raw2[0:64, :], raw[64:128, :])
            nc.vector.tensor_copy(raw2[64:128, :], raw[0:64, :])
            p_ = rtmp.tile([128, 512], bf, tag="p")
            q_ = rtmp.tile([128, 512], bf, tag="q")
            # tblA = [cos; cos], tblB = [-sin; sin] (host-baked signs)
            nc.vector.tensor_tensor(p_[:], raw[:], tblA[:, tsl], mult)
            nc.vector.tensor_tensor(q_[:], raw2[:], tblB[:, tsl], mult)
            nc.vector.tensor_tensor(qk[:, qkcb, tsl], p_[:], q_[:], add)

        def v_group(tb, pool, ptag):
            # computes only this core's parity-half of the v channels; the
            # pair AllGather fills vsb[:, :, 0:256] on both cores
            xt = xts[tb // 4]
            to = (tb % 4) * 128
            ps = pool.tile([128, 512], f32, tag=ptag, name="psv")
            for k in range(KB):
                nc.tensor.matmul(ps[:, 0:128], xt[:, k, to:to + 128], wv[:, k],
                                 start=(k == 0), stop=(k == KB - 1),
                                 skip_group_check=True)
                yield
            nc.scalar.copy(vsb[:, tb, 0:128], ps[:, 0:128])

        def do_collective(b):
            # pair {2c, 2c+1} exchange: my K stream + my half of V channels
            bsl = slice(b * T, (b + 1) * T)
            tbs = slice(b * 8, b * 8 + 8)
            ccin = dram.tile([128, 2048], bf, tag="ccin", name=f"ccin{b}")
            ccout = dram.tile([2, 128, 2048], bf, tag="ccout", name=f"ccout{b}")
            nc.gpsimd.dma_start(ccin[:, 0:1024], qk[:, 4, bsl])
            nc.gpsimd.dma_start(ccin[:, 1024:2048], vsb[:, tbs, 0:128])
            nc.gpsimd.collective_compute(
                "AllGather", mybir.AluOpType.bypass,
                replica_groups=[[0, 1], [2, 3], [4, 5], [6, 7]],
                ins=[ccin[:].opt()], outs=[ccout[:].opt()])
            nc.gpsimd.dma_start(qk[:, 4, bsl], ccout[0, :, 0:1024])
            nc.gpsimd.dma_start(qk[:, 5, bsl], ccout[1, :, 0:1024])
            nc.gpsimd.dma_start(vsb[:, tbs, 0:128], ccout[0, :, 1024:2048])
            nc.gpsimd.dma_start(vsb[:, tbs, 128:256], ccout[1, :, 1024:2048])

        def oproj_tb(tb):
            for np_ in range(2):
                pso = [pop.tile([128, 512], f32, tag="op", name=f"pso{n}")
                       for n in range(2)]
                for cb in range(4):
                    for ni, n in enumerate((2 * np_, 2 * np_ + 1)):
                        nc.tensor.matmul(pso[ni][:],
                                         osbT[:, cb, tb * 128:(tb + 1) * 128],
                                         wo_sb[:, cb, n * 512:(n + 1) * 512],
                                         start=(cb == 0), stop=(cb == 3),
                                         skip_group_check=True)
                        yield
                for ni, n in enumerate((2 * np_, 2 * np_ + 1)):
                    so = stage.tile([128, 512], bf, tag="so")
                    nc.scalar.copy(so[:], pso[ni][:])
                    nc.sync.dma_start(out_d[tb * 128:(tb + 1) * 128,
                                            n * 512:(n + 1) * 512], so[:])

        # ---- phase 1: projections; K/V halves feed pair AllGathers -------
        for ts in range(4):
            ropes = (4, 0, 1, 2, 3) if ts < 3 else (4,)
            for qkcb in ropes:
                for _ in rope_group(qkcb, ts, pmm, "mm"):
                    pass
            for tb in range(ts * 4, ts * 4 + 4):
                for _ in v_group(tb, pmm, "mm"):
                    pass
            if ts == 1:
                do_collective(0)    # b0 K/V exchange overlaps ts2/ts3 proj
            if ts < 2:
                load_x(ts + 2)      # recycles xt[ts] buffer: all reads done
        do_collective(1)

        # ---- phase 2: attention with pipelined filler --------------------
        def scores_part(h, b, J, fill):
            nk = 4 * (J + 1)
            qsl = slice(b * T + J * 512, b * T + (J + 1) * 512)
            e1 = epool.tile([128, nk, 512], bf, tag="e1", name="e1")
            e2 = epool.tile([128, nk, 512], bf, tag="e2", name="e2")
            for s, et in ((0, e1), (1, e2)):
                qblk = qk[:, 2 * h + s, qsl]
                for i in range(nk):
                    lo = max(0, i - 4 * J) * 128
                    ksl = slice(b * T + i * 128, b * T + (i + 1) * 128)
                    ps = pmm.tile([128, 512], f32, tag="mm", name="pssc")
                    nc.tensor.matmul(ps[:, lo:512], qk[:, 4 + s, ksl],
                                     qblk[:, lo:512], start=True, stop=True,
                                     skip_group_check=True)
                    next(fill, None)
                    if lo == 0:
                        next(fill, None)
                    nc.scalar.activation(et[:, i, lo:512], ps[:, lo:512], Exp)
                    if i >= 4 * J:      # diagonal sub-block masking
                        nc.vector.tensor_tensor(et[:, i, lo:lo + 128],
                                                et[:, i, lo:lo + 128],
                                                tri[:], mult)
            return e1, e2

        def av_part(h, b, J, e1, e2, o2s, fill):
            drained = []
            for jl in range(4):     # phase A: AV matmuls + fast u-psum drain
                jg = 4 * J + jl
                qq = slice(jl * 128, (jl + 1) * 128)
                u1f = pav.tile([128, 512], f32, tag="u1", name="u1")
                u2f = pav.tile([128, 512], f32, tag="u2", name="u2")
                u1 = u1f[:, 0:258]
                u2 = u2f[:, 0:258]
                for i in range(jg + 1):
                    vt = vsb[:, b * QB + i, :]
                    nc.tensor.matmul(u1, e1[:, i, qq], vt,
                                     start=(i == 0), stop=(i == jg),
                                     skip_group_check=True)
                    nc.tensor.matmul(u2, e2[:, i, qq], vt,
                                     start=(i == 0), stop=(i == jg),
                                     skip_group_check=True)
                next(fill, None)
                next(fill, None)
                r1 = spool.tile([128, 1], f32, tag="r1", name=f"r1_{jl}")
                r2 = spool.tile([128, 1], f32, tag="r2", name=f"r2_{jl}")
                nc.vector.reciprocal(r1[:], u1[:, 256:257])
                nc.vector.reciprocal(r2[:], u2[:, 257:258])   # = -lam/z2
                u1b = opool.tile([128, 256], bf, tag="u1b", name=f"u1b_{jl}")
                u2b = opool.tile([128, 256], bf, tag="u2b", name=f"u2b_{jl}")
                nc.scalar.copy(u1b[:], u1[:, 0:256])      # Act
                nc.vector.tensor_copy(u2b[:], u2[:, 0:256])  # DVE
                drained.append((jg, r1, r2, u1b, u2b))
            for jl, (jg, r1, r2, u1b, u2b) in enumerate(drained):
                # phase B: combine streams + GroupNorm over 256 channels
                o1 = opool.tile([128, 256], bf, tag="o1", name=f"o1_{jl}")
                nc.vector.tensor_scalar_mul(o1[:], u1b[:], r1[:])
                nc.vector.scalar_tensor_tensor(o1[:], u2b[:], r2[:],
                                               o1[:], mult, add)
                st6 = spool.tile([128, 6], f32, tag="st6")
                mv = spool.tile([128, 2], f32, tag="mv")
                nc.vector.bn_stats(st6[:], o1[:])
                nc.vector.bn_aggr(mv[:], st6[:])
                lnv = spool.tile([128, 1], f32, tag="lnv")
                rstd = spool.tile([128, 1], f32, tag="rstd")
                # rstd = (var+eps)^-0.5 via ln/exp (same act table as Exp)
                nc.scalar.activation(lnv[:], mv[:, 1:2], Ln, bias=epst[:])
                nc.scalar.activation(rstd[:], lnv[:], Exp, scale=-0.5)
                o2 = opool2.tile([128, 256], bf, tag="o2", name=f"o2_{jl}")
                nc.vector.tensor_scalar(o2[:], o1[:], mv[:, 0:1], rstd[:],
                                        sub, mult)
                o2s.append((h, jg, o2))

        def unit_chain(units):
            for u in units:
                yield from u

        def transpose_block(b, h2, jg, o2, half):
            pst = pop.tile([128, 512], bf, tag="op", name="pst")
            nc.tensor.transpose(pst[:, 0:128],
                                o2[:, half * 128:(half + 1) * 128],
                                ident[:])
            nc.vector.tensor_copy(
                osbT[:, 2 * h2 + half,
                     b * T + jg * 128:b * T + (jg + 1) * 128],
                pst[:, 0:128])

        def do_group(b, J, units, tail_tbs=None):
            fill = unit_chain(units)
            o2s = []
            for h in range(HPC):
                e1, e2 = scores_part(h, b, J, fill)
                av_part(h, b, J, e1, e2, o2s, fill)
            for _ in fill:          # flush remaining filler work
                pass
            # transposes (q,ch)->(ch,q) into resident osbT
            if tail_tbs is None:
                for h2, jg, o2 in o2s:
                    for half in range(2):
                        transpose_block(b, h2, jg, o2, half)
            else:
                # last group: per-q-block, both heads, then that token
                # block's o-projection immediately (shrinks the end tail)
                byjl = {(h2, jg): o2 for h2, jg, o2 in o2s}
                for jl in range(4):
                    jg = 4 * J + jl
                    for h2 in range(HPC):
                        for half in range(2):
                            transpose_block(b, h2, jg, byjl[(h2, jg)], half)
                    for _ in oproj_tb(tail_tbs[jl]):
                        pass

        # q-ropes for ts3: two inline (slack for the b0 collective to land),
        # two as filler inside the first attention group
        for _ in rope_group(0, 3, pmm, "mm"):
            pass
        for _ in rope_group(1, 3, pmm, "mm"):
            pass
        do_group(0, 0, [rope_group(c, 3, pop, "op") for c in (2, 3)])
        do_group(0, 1, [oproj_tb(tb) for tb in (0, 1, 2, 3)])
        do_group(1, 0, [oproj_tb(tb) for tb in (4, 5, 6, 7)])
        do_group(1, 1, [oproj_tb(tb) for tb in (8, 9, 10, 11)],
                 tail_tbs=(12, 13, 14, 15))


# ------------------------- host side  ------------------------------------

_ROPE_PERM = np.concatenate([np.arange(0, DH, 2), np.arange(1, DH, 2)])


def _prep(x, Wq, Wk, Wv, Wo, lambda_q1, lambda_k1, lambda_q2, lambda_k2,
          gn_weight, gn_bias, pos):
    lam = float(np.exp(np.sum(lambda_q1 * lambda_k1))
                - np.exp(np.sum(lambda_q2 * lambda_k2)) + LAMBDA_INIT)
    if abs(lam) < 1e-6:             # keep the baked -1/lam column finite
        lam = 1e-6 if lam >= 0 else -1e-6
    scale = DH ** -0.5

    posf = pos.astype(np.float64)
    inv = 1.0 / (ROPE_BASE ** (np.arange(0, DH, 2, dtype=np.float32) / DH))
    freqs = (posf[:, None] * inv[None, :].astype(np.float64)).astype(np.float32)
    cosv = np.cos(freqs).T          # (64, T)
    sinv = np.sin(freqs).T
    cost = np.tile(cosv, (1, B))
    sint = np.tile(sinv, (1, B))
    tblA = np.ascontiguousarray(
        np.concatenate([cost, cost], axis=0)).astype(bf16)   # [cos; cos]
    tblB = np.ascontiguousarray(
        np.concatenate([-sint, sint], axis=0)).astype(bf16)  # [-sin; sin]

    # x transposed: (D, B*T) -> (TSB, 128, KB, 512) ts-major contiguous
    xT = x.reshape(TOK, D).T.astype(np.float32)
    x3 = xT.reshape(KB, 128, TSB, 512).transpose(2, 1, 0, 3)
    xT3 = np.ascontiguousarray(x3).astype(bf16)

    # Wq: (H,2,DH,D), rope-permute DH, fold score scale
    Wq4 = (Wq.reshape(H, 2, DH, D)[:, :, _ROPE_PERM, :] * scale).astype(np.float32)
    Wk4 = Wk.reshape(KV, 2, DH, D)[:, :, _ROPE_PERM, :].astype(np.float32)
    Wv3 = Wv.reshape(KV, 2 * DH, D).astype(np.float32)

    s1 = 1.0 - LAMBDA_INIT
    Wo_f = (Wo * (gn_weight * s1)[None, :]).astype(np.float32)   # (D, 4096)
    bias_out = (gn_bias * s1).astype(np.float32) @ Wo.T.astype(np.float32)

    def to_sb(w2d, cols):           # (D, cols) -> (128, KB, cols) bf16
        return np.ascontiguousarray(
            w2d.reshape(KB, 128, cols).transpose(1, 0, 2)).astype(bf16)

    in_maps = []
    for c in range(NCORES):
        p = c % 2          # parity: which K stream / V half this core computes
        wqT = Wq4[2 * c:2 * c + 2].reshape(512, D).T          # (D, 512)
        wkhT = Wk4[c // 2, p].T                               # (D, 128)
        wvhT = Wv3[c // 2].T[:, p * 128:(p + 1) * 128]        # (D, 128)
        woT = Wo_f[:, 512 * c:512 * c + 512].T                # (512 ch, D out)
        wo3 = np.ascontiguousarray(
            woT.reshape(4, 128, D).transpose(1, 0, 2)).astype(bf16)
        wq_stack = np.stack([to_sb(wqT[:, cb * 128:(cb + 1) * 128], 128)
                             for cb in range(4)])             # (4,128,KB,128)
        in_maps.append({
            "xT": xT3,
            "wqT": wq_stack,
            "wkh": to_sb(wkhT, 128),
            "wvh": to_sb(wvhT, 128),
            "woT": wo3,
            "tbl": tblA,
            "tbl2": tblB,
        })
    return lam, in_maps, bias_out


LAST_RESULT = None


def kernel(**inputs):
    global LAST_RESULT
    inputs = {k: np.asarray(v) for k, v in inputs.items()}
    lam, in_maps, bias_out = _prep(**inputs)
    nc = build_program(lam)
    res = run_bass_kernel_spmd(nc, in_maps, core_ids=list(range(NCORES)))
    LAST_RESULT = res
    out = np.zeros((TOK, D), np.float32)
    for c in range(NCORES):
        out += res.results[c]["out"].astype(np.float32)
    out += bias_out[None, :]
    return out.reshape(B, T, D).astype(np.float32)


if __name__ == "__main__":
    import reference
    ins = {k: np.asarray(v) for k, v in reference.setup_inputs().items()}
    got = kernel(**ins)
    exp = np.asarray(reference.reference(**ins))
    rel = np.linalg.norm(got - exp) / np.linalg.norm(exp)
    print("rel err:", rel)
